# revision 18
# baseline (speedup 1.0000x reference)
"""FlowNet-style patch correlation (KERNEL=1, MAX_DISP=4, pad=4) on 8 trn2
NeuronCores — Gram-matmul formulation.

Per core (2 batches), per pixel row y:
  - 5 matmuls (4 quadrants of x 0..127 via tile_position, + side x 128..159
    packed 4 consecutive y per bank): lhsT = I1[:, y, 32q:32q+32] fp16,
    rhs = I2pad[:, y:y+9, 32q:32q+40] (all 9 dy in one instruction) ->
    PSUM [128, 360] Gram G[(q,u), (dy,j)] = sum_c I1[c,y,x]*I2[c,y+dy-4,x'].
  - scaled engine copy per bank, reading PSUM in (j, dy) order so the dense
    fp16 SBUF stage gst[p, slot*360 + j*9+dy] is band-interleaved.
  - per-u extraction DMA (partition stride = exactly 32 rows -> BIR-legal)
    gathers the 81 useful values of pixel x = 32q+u (at in-row offset u*9)
    contiguously: band[x, slot*81 + t], t = (dx+4)*9 + (dy+4).
  - PE transpose [128,81] -> PSUM [81,128] per y-slot, engine copy into a
    fp16 staging tile, 9 gpsimd cast-DMAs (one per dx) write out fp32.
Pipelined in 32-row segments with 3-deep gst/band rings: extraction of ring
N overlaps compute of N+1 and N+2; previous ring's transposes interleave
into the current matmul loop with an 8-iteration delay.
"""

import sys
from contextlib import ExitStack

import numpy as np

for _p in ("/opt/trn_rl_repo", "/root/.axon_site/_ro/trn_rl_repo"):
    if _p not in sys.path:
        sys.path.insert(0, _p)

import concourse.bass as bass
import concourse.tile as tile
from concourse import mybir
from concourse.bass_utils import run_bass_kernel_spmd
from concourse.masks import make_identity

B, C, H, W = 16, 128, 96, 160
MD = 4
ND = 2 * MD + 1            # 9
D = ND * ND                # 81
N_CORES = 8
BPC = B // N_CORES         # 2
HP, WP = H + 2 * MD, W + 2 * MD    # 104 x 168
NQ = 4                     # main quadrants (x 0..127)
QW = 32                    # quadrant width
WIN = QW + 2 * MD          # 40 rhs cols per (dy, quadrant)
GF = ND * WIN              # 360 psum free size per bank
TT = 48                    # ys per ring segment
NT = H // TT               # 3 segments per batch
NG = TT // 4               # side groups per segment = 8
TR = TT + 2 * MD           # padded i2 rows per segment tensor = 40

f32 = mybir.dt.float32
f16 = mybir.dt.float16

GROW = TT * GF             # gst main row elems per segment (32*360)
SROW = NG * GF             # gst side row (8*360)
BROW = TT * D              # band main row (32*81)
SBROW = NG * D             # band side row (8*81)


def _split_waits(nc, limit=1):
    n = 0
    for fn in nc.m.functions:
        for blk in fn.blocks:
            out = []
            for inst in blk.instructions:
                si = inst.sync_info
                if si is not None and si.on_wait and len(si.on_wait) > limit:
                    waits = list(si.on_wait)
                    for w in waits[:-limit]:
                        out.append(mybir.InstEventSemaphore(
                            name=nc.get_next_instruction_name(),
                            sync_info=mybir.SyncInfo(on_wait=[w], on_update=[]),
                            engine=inst.engine,
                            ins=[], outs=[],
                        ))
                        n += 1
                    inst.sync_info = mybir.SyncInfo(
                        on_wait=waits[-limit:], on_update=list(si.on_update or []))
                out.append(inst)
            blk.instructions[:] = out
    return n


def _ap(t_ap, dims, offset):
    c = t_ap.copy()
    c.ap = mybir.VecI64Pair(dims)
    c.offset = offset
    return c


class _Seg:
    pass


def _corr_kernel(ctx, tc, out4, in1, in2):
    nc = tc.nc
    const_pool = ctx.enter_context(tc.tile_pool(name="const", bufs=1))
    in_pool = ctx.enter_context(tc.tile_pool(name="in", bufs=1))
    gstm_pool = ctx.enter_context(tc.tile_pool(name="gstm", bufs=2))
    gsts_pool = ctx.enter_context(tc.tile_pool(name="gsts", bufs=2))
    bndm_pool = ctx.enter_context(tc.tile_pool(name="bndm", bufs=1))
    bnds_pool = ctx.enter_context(tc.tile_pool(name="bnds", bufs=1))
    trim_pool = ctx.enter_context(tc.tile_pool(name="trim", bufs=1))
    tris_pool = ctx.enter_context(tc.tile_pool(name="tris", bufs=1))
    stg_pool = ctx.enter_context(tc.tile_pool(name="stg", bufs=2))
    psm_pool = ctx.enter_context(tc.tile_pool(name="psm", bufs=3, space="PSUM"))
    pss_pool = ctx.enter_context(tc.tile_pool(name="pss", bufs=2, space="PSUM"))
    pt_pool = ctx.enter_context(tc.tile_pool(name="pt", bufs=2, space="PSUM"))
    pts_pool = ctx.enter_context(tc.tile_pool(name="pts", bufs=1, space="PSUM"))

    ident = const_pool.tile([128, 128], f16, name="ident")
    make_identity(nc, ident[:, :])

    # double-buffered per-segment inputs; i2 segments carry the 8-row halo
    i1h = [in_pool.tile([C, TT * W], f16, tag=f"i1h{t}", name=f"i1h{t}")
           for t in range(2)]
    i2h = [in_pool.tile([C, TR * WP], f16, tag=f"i2h{t}", name=f"i2h{t}")
           for t in range(2)]
    i1h3 = [t[:, :].rearrange("p (y x) -> p y x", y=TT) for t in i1h]
    i2h3 = [t[:, :].rearrange("p (y x) -> p y x", y=TR) for t in i2h]
    for t in range(2):
        nc.vector.memset(i2h3[t][:, :, 0:MD], 0.0)
        nc.vector.memset(i2h3[t][:, :, MD + W:WP], 0.0)

    inv_c = 1.0 / C

    def load_seg(b, tt, buf):
        for c0, c1 in ((0, 16), (16, TT)):
            nc.gpsimd.dma_start(
                out=i1h3[buf][:, c0:c1, :],
                in_=in1[b, :, TT * tt + c0:TT * tt + c1, :])
        ylo, yhi = TT * tt - MD, TT * (tt + 1) + MD
        # zero the halo rows that fall outside the image
        if ylo < 0:
            nc.vector.memset(i2h3[buf][:, 0:-ylo, MD:MD + W], 0.0)
        if yhi > H:
            nc.vector.memset(i2h3[buf][:, TR - (yhi - H):TR, MD:MD + W], 0.0)
        for c0, c1 in ((ylo, ylo + 28), (ylo + 28, yhi)):
            clo, chi = max(c0, 0), min(c1, H)
            if clo < chi:
                nc.gpsimd.dma_start(
                    out=i2h3[buf][:, clo - ylo:chi - ylo, MD:MD + W],
                    in_=in2[b, :, clo:chi, :])

    def transpose_step(pv, i):
        pt = pt_pool.tile([D, 128], f16, tag="pt",
                          name=f"pt_{pv.b}_{pv.tt}_{i}")
        # trin layout: trin[x, t*TT + yslot] -> single-stride weights AP
        src_m = _ap(pv.trin[:, :], [[BROW, 128], [TT, D]], i)
        nc.tensor.transpose(pt[:, :], src_m, ident[:, :])
        if i % 3 == 2:
            nc.scalar.copy(pv.stgt[0:D, i * W:i * W + NQ * QW], pt[:, :])
        else:
            nc.vector.tensor_copy(out=pv.stgt[0:D, i * W:i * W + NQ * QW],
                                  in_=pt[:, :])
        if i % 4 == 3:
            gi = i // 4
            pts = pts_pool.tile([D, 128], f16, tag="pts",
                                name=f"pts_{pv.b}_{pv.tt}_{gi}")
            src_s = _ap(pv.trins[:, :], [[SBROW, 128], [NG, D]], gi)
            nc.tensor.transpose(pts[:, :], src_s, ident[:, :])
            dsts = _ap(pv.stgt[:, :], [[TT * W, D], [W, 4], [1, QW]],
                       (gi * 4) * W + NQ * QW)
            if gi % 3 == 2:
                nc.scalar.copy(dsts, pts[:, :])
            else:
                nc.vector.tensor_copy(out=dsts, in_=pts[:, :])

    def finish_seg(pv):
        # stg row t = k*9+dy maps to output d = dy*9+k
        for k in range(ND):
            nc.gpsimd.dma_start(
                out=out4[pv.b, k::ND, TT * pv.tt:TT * (pv.tt + 1), :],
                in_=pv.stgt[k * ND:(k + 1) * ND, :])

    # first load: small lead chunks so the first matmuls start early
    nc.gpsimd.dma_start(out=i2h3[0][:, MD:16, MD:MD + W],
                        in_=in2[0, :, 0:12, :])
    nc.gpsimd.dma_start(out=i1h3[0][:, 0:8, :], in_=in1[0, :, 0:8, :])
    nc.vector.memset(i2h3[0][:, 0:MD, MD:MD + W], 0.0)
    nc.gpsimd.dma_start(out=i2h3[0][:, 16:TR, MD:MD + W],
                        in_=in2[0, :, 12:TT + MD, :])
    nc.gpsimd.dma_start(out=i1h3[0][:, 8:TT, :], in_=in1[0, :, 8:TT, :])
    _first_loaded = True
    prev = None
    segs = [(b, tt) for b in range(BPC) for tt in range(NT)]
    for si, (b, tt) in enumerate(segs):
        if si + 1 < len(segs):
            load_seg(*segs[si + 1], (si + 1) % 2)

        gstm = gstm_pool.tile([C, GROW], f16, tag="gstm",
                              name=f"gstm_{b}_{tt}")
        gsts = gsts_pool.tile([C, SROW], f16, tag="gsts",
                              name=f"gsts_{b}_{tt}")
        i13, i2p3 = i1h3[si % 2], i2h3[si % 2]

        pss = None
        for i in range(TT):
            if prev is not None and i >= 14:
                transpose_step(prev, i - 14)
            ps = psm_pool.tile([128, GF], f32, tag="psm",
                               name=f"psm_{b}_{tt}_{i}")
            # rhs streams (j outer, dy inner) so PSUM col n = j*ND + dy
            i2t = i2h[si % 2][:, :]
            for q in range(NQ):
                nc.tensor.matmul(
                    out=ps[QW * q:QW * (q + 1), :],
                    lhsT=i13[:, i, QW * q:QW * (q + 1)],
                    rhs=_ap(i2t, [[TR * WP, 128], [1, WIN], [WP, ND]],
                            i * WP + QW * q),
                    start=True, stop=True,
                    tile_position=(0, QW * q),
                )
            m = i % 4
            if m == 0:
                pss = pss_pool.tile([128, GF], f32, tag="pss",
                                    name=f"pss_{b}_{tt}_{i}")
            nc.tensor.matmul(
                out=pss[QW * m:QW * (m + 1), :],
                lhsT=i13[:, i, NQ * QW:W],
                rhs=_ap(i2t, [[TR * WP, 128], [1, WIN], [WP, ND]],
                        i * WP + NQ * QW),
                start=True, stop=True,
                tile_position=(0, QW * m),
            )
            # contiguous PSUM read -> banded gst[x, j*(ND*TT) + yslot*ND + dy]
            # (9-elem contiguous write runs; pixel band still one big run)
            src_i = _ap(ps[:, :], [[GF, 128], [1, GF]], 0)
            dst_i = _ap(gstm[:, :], [[GROW, 128], [ND * TT, WIN], [1, ND]],
                        i * ND)
            if i % 3 != 2:
                nc.vector.tensor_scalar_mul(dst_i, src_i, inv_c)
            else:
                nc.scalar.mul(dst_i, src_i, inv_c)
            if m == 3:
                gi = i // 4
                src_s = _ap(pss[:, :], [[GF, 128], [1, GF]], 0)
                dst_s = _ap(gsts[:, :], [[SROW, 128], [ND * NG, WIN], [1, ND]],
                            gi * ND)
                if gi % 3 == 2:
                    nc.scalar.mul(dst_s, src_s, inv_c)
                else:
                    nc.vector.tensor_scalar_mul(dst_s, src_s, inv_c)

        if prev is not None:
            for j in range(TT - 14, TT):
                transpose_step(prev, j)
            finish_seg(prev)

        # per-u band extraction (partition stride = whole rows only)
        cur = _Seg()
        cur.b, cur.tt = b, tt
        cur.bnd = bndm_pool.tile([C, BROW], f16, tag="bndm",
                                 name=f"bnd_{b}_{tt}")
        cur.bnds = bnds_pool.tile([C, SBROW], f16, tag="bnds",
                                  name=f"bnds_{b}_{tt}")
        cur.stgt = stg_pool.tile([96, TT * W], f16, tag="stg",
                                 name=f"stg_{b}_{tt}")
        # banded gst: pixel x=32q+u needs j in [u, u+9) -> one contiguous
        # run of ND*(ND*TT) = BROW elems per partition (4 big descriptors)
        for u in range(QW):
            eng = nc.sync if u % 2 == 0 else nc.scalar
            eng.dma_start(
                out=_ap(cur.bnd[:, :], [[QW * BROW, 4], [1, BROW]], u * BROW),
                in_=_ap(gstm[:, :], [[QW * GROW, 4], [1, BROW]],
                        u * GROW + u * ND * TT),
            )
        for u in range(QW):
            nc.gpsimd.dma_start(
                out=_ap(cur.bnds[:, :], [[QW * SBROW, 4], [1, SBROW]],
                        u * SBROW),
                in_=_ap(gsts[:, :], [[QW * SROW, 4], [1, SBROW]],
                        u * SROW + u * ND * NG),
            )
        # reorder (dxi, yslot, dy) -> (t, yslot): scattered 18B reads,
        # contiguous TT-elem write runs (keeps DVE write path fast)
        cur.trin = trim_pool.tile([C, BROW], f16, tag="trim",
                                  name=f"trin_{b}_{tt}")
        cur.trins = tris_pool.tile([C, SBROW], f16, tag="trins",
                                   name=f"trins_{b}_{tt}")
        for e in range(3):
            dst_r = _ap(cur.trin[:, :], [[BROW, 128], [ND * TT, 3], [TT, ND],
                                         [1, TT]], e * 3 * ND * TT)
            src_r = _ap(cur.bnd[:, :], [[BROW, 128], [ND * TT, 3], [1, ND],
                                        [ND, TT]], e * 3 * ND * TT)
            if e == 1:
                nc.scalar.copy(dst_r, src_r)
            else:
                nc.vector.tensor_copy(out=dst_r, in_=src_r)
        nc.scalar.copy(
            _ap(cur.trins[:, :], [[SBROW, 128], [ND * NG, ND], [NG, ND],
                                  [1, NG]], 0),
            _ap(cur.bnds[:, :], [[SBROW, 128], [ND * NG, ND], [1, ND],
                                 [ND, NG]], 0),
        )
        prev = cur

    for i in range(TT):
        transpose_step(prev, i)
    finish_seg(prev)


_NC_CACHE = {}


def _build_module():
    if "nc" in _NC_CACHE:
        return _NC_CACHE["nc"]
    nc = bass.Bass("TRN2", target_bir_lowering=False, debug=False)
    in1 = nc.dram_tensor("input1", [BPC, C, H, W], f32, kind="ExternalInput").ap()
    in2 = nc.dram_tensor("input2", [BPC, C, H, W], f32, kind="ExternalInput").ap()
    out = nc.dram_tensor("output", [BPC, D, H, W], f32, kind="ExternalOutput").ap()
    with tile.TileContext(nc) as tc:
        with ExitStack() as ctx:
            _corr_kernel(ctx, tc, out, in1, in2)
    _split_waits(nc)
    _NC_CACHE["nc"] = nc
    return nc


def kernel(input1: np.ndarray, input2: np.ndarray, **trace_kwargs) -> np.ndarray:
    input1 = np.ascontiguousarray(input1, dtype=np.float32)
    input2 = np.ascontiguousarray(input2, dtype=np.float32)
    assert input1.shape == (B, C, H, W) and input2.shape == (B, C, H, W)

    nc = _build_module()
    in_maps = []
    for k in range(N_CORES):
        sl = slice(k * BPC, (k + 1) * BPC)
        in_maps.append({
            "input1": input1[sl],
            "input2": input2[sl],
        })
    res = run_bass_kernel_spmd(nc, in_maps, list(range(N_CORES)), **trace_kwargs)
    outs = [res.results[k]["output"] for k in range(N_CORES)]
    full = np.concatenate(outs, axis=0)
    if trace_kwargs:
        kernel.last_results = res
    return full



# revision 26
# speedup vs baseline: 1.7013x; 1.7013x over previous
"""FlowNet-style patch correlation (KERNEL=1, MAX_DISP=4, pad=4) on 8 trn2
NeuronCores — Gram-matmul formulation.

Per core (2 batches), per pixel row y:
  - 5 matmuls (4 quadrants of x 0..127 via tile_position, + side x 128..159
    packed 4 consecutive y per bank): lhsT = I1[:, y, 32q:32q+32] fp16,
    rhs = I2pad[:, y:y+9, 32q:32q+40] (all 9 dy in one instruction) ->
    PSUM [128, 360] Gram G[(q,u), (dy,j)] = sum_c I1[c,y,x]*I2[c,y+dy-4,x'].
  - scaled engine copy per bank, reading PSUM in (j, dy) order so the dense
    fp16 SBUF stage gst[p, slot*360 + j*9+dy] is band-interleaved.
  - per-u extraction DMA (partition stride = exactly 32 rows -> BIR-legal)
    gathers the 81 useful values of pixel x = 32q+u (at in-row offset u*9)
    contiguously: band[x, slot*81 + t], t = (dx+4)*9 + (dy+4).
  - PE transpose [128,81] -> PSUM [81,128] per y-slot, engine copy into a
    fp16 staging tile, 9 gpsimd cast-DMAs (one per dx) write out fp32.
Pipelined in 32-row segments with 3-deep gst/band rings: extraction of ring
N overlaps compute of N+1 and N+2; previous ring's transposes interleave
into the current matmul loop with an 8-iteration delay.
"""

import sys
from contextlib import ExitStack

import numpy as np

for _p in ("/opt/trn_rl_repo", "/root/.axon_site/_ro/trn_rl_repo"):
    if _p not in sys.path:
        sys.path.insert(0, _p)

import concourse.bass as bass
import concourse.tile as tile
from concourse import mybir
from concourse.bass_utils import run_bass_kernel_spmd
from concourse.masks import make_identity

B, C, H, W = 16, 128, 96, 160
MD = 4
ND = 2 * MD + 1            # 9
D = ND * ND                # 81
N_CORES = 8
BPC = B // N_CORES         # 2
HP, WP = H + 2 * MD, W + 2 * MD    # 104 x 168
NQ = 4                     # main quadrants (x 0..127)
QW = 32                    # quadrant width
WIN = QW + 2 * MD          # 40 rhs cols per (dy, quadrant)
GF = ND * WIN              # 360 psum free size per bank
TT = 48                    # ys per ring segment
NT = H // TT               # 3 segments per batch
NG = TT // 4               # side groups per segment = 8
TR = TT + 2 * MD           # padded i2 rows per segment tensor = 40

f32 = mybir.dt.float32
f16 = mybir.dt.float16

GROW = TT * GF             # gst main row elems per segment (32*360)
SROW = NG * GF             # gst side row (8*360)
BROW = TT * D              # band main row (32*81)
SBROW = NG * D             # band side row (8*81)


def _split_waits(nc, limit=1):
    n = 0
    for fn in nc.m.functions:
        for blk in fn.blocks:
            out = []
            for inst in blk.instructions:
                si = inst.sync_info
                if si is not None and si.on_wait and len(si.on_wait) > limit:
                    waits = list(si.on_wait)
                    for w in waits[:-limit]:
                        out.append(mybir.InstEventSemaphore(
                            name=nc.get_next_instruction_name(),
                            sync_info=mybir.SyncInfo(on_wait=[w], on_update=[]),
                            engine=inst.engine,
                            ins=[], outs=[],
                        ))
                        n += 1
                    inst.sync_info = mybir.SyncInfo(
                        on_wait=waits[-limit:], on_update=list(si.on_update or []))
                out.append(inst)
            blk.instructions[:] = out
    return n


def _ap(t_ap, dims, offset):
    c = t_ap.copy()
    c.ap = mybir.VecI64Pair(dims)
    c.offset = offset
    return c


class _Seg:
    pass


def _corr_kernel(ctx, tc, out4, in1, in2):
    nc = tc.nc
    const_pool = ctx.enter_context(tc.tile_pool(name="const", bufs=1))
    in_pool = ctx.enter_context(tc.tile_pool(name="in", bufs=1))
    gstm_pool = ctx.enter_context(tc.tile_pool(name="gstm", bufs=2))
    gsts_pool = ctx.enter_context(tc.tile_pool(name="gsts", bufs=2))
    bndm_pool = ctx.enter_context(tc.tile_pool(name="bndm", bufs=1))
    bnds_pool = ctx.enter_context(tc.tile_pool(name="bnds", bufs=1))
    trim_pool = ctx.enter_context(tc.tile_pool(name="trim", bufs=1))
    tris_pool = ctx.enter_context(tc.tile_pool(name="tris", bufs=1))
    stg_pool = ctx.enter_context(tc.tile_pool(name="stg", bufs=2))
    psm_pool = ctx.enter_context(tc.tile_pool(name="psm", bufs=3, space="PSUM"))
    pss_pool = ctx.enter_context(tc.tile_pool(name="pss", bufs=2, space="PSUM"))
    pt_pool = ctx.enter_context(tc.tile_pool(name="pt", bufs=2, space="PSUM"))
    pts_pool = ctx.enter_context(tc.tile_pool(name="pts", bufs=1, space="PSUM"))

    ident = const_pool.tile([128, 128], f16, name="ident")
    make_identity(nc, ident[:, :])

    # double-buffered per-segment inputs; i2 segments carry the 8-row halo
    i1h = [in_pool.tile([C, TT * W], f16, tag=f"i1h{t}", name=f"i1h{t}")
           for t in range(2)]
    i2h = [in_pool.tile([C, TR * WP], f16, tag=f"i2h{t}", name=f"i2h{t}")
           for t in range(2)]
    i1h3 = [t[:, :].rearrange("p (y x) -> p y x", y=TT) for t in i1h]
    i2h3 = [t[:, :].rearrange("p (y x) -> p y x", y=TR) for t in i2h]
    for t in range(2):
        nc.vector.memset(i2h3[t][:, :, 0:MD], 0.0)
        nc.vector.memset(i2h3[t][:, :, MD + W:WP], 0.0)

    inv_c = 1.0 / C

    def load_seg(b, tt, buf):
        for c0, c1 in ((0, 16), (16, TT)):
            nc.gpsimd.dma_start(
                out=i1h3[buf][:, c0:c1, :],
                in_=in1[b, :, TT * tt + c0:TT * tt + c1, :])
        ylo, yhi = TT * tt - MD, TT * (tt + 1) + MD
        # zero the halo rows that fall outside the image
        if ylo < 0:
            nc.vector.memset(i2h3[buf][:, 0:-ylo, MD:MD + W], 0.0)
        if yhi > H:
            nc.vector.memset(i2h3[buf][:, TR - (yhi - H):TR, MD:MD + W], 0.0)
        for c0, c1 in ((ylo, ylo + 28), (ylo + 28, yhi)):
            clo, chi = max(c0, 0), min(c1, H)
            if clo < chi:
                nc.gpsimd.dma_start(
                    out=i2h3[buf][:, clo - ylo:chi - ylo, MD:MD + W],
                    in_=in2[b, :, clo:chi, :])

    def transpose_step(pv, i):
        pt = pt_pool.tile([D, 128], f16, tag="pt",
                          name=f"pt_{pv.b}_{pv.tt}_{i}")
        # trin layout: trin[x, yslot*D + t] -> contiguous weights slice
        nc.tensor.transpose(pt[:, :], pv.trin[:, i * D:(i + 1) * D],
                            ident[:, :])
        if i % 3 == 2:
            nc.scalar.copy(pv.stgt[0:D, i * W:i * W + NQ * QW], pt[:, :])
        else:
            nc.vector.tensor_copy(out=pv.stgt[0:D, i * W:i * W + NQ * QW],
                                  in_=pt[:, :])
        if i % 4 == 3:
            gi = i // 4
            pts = pts_pool.tile([D, 128], f16, tag="pts",
                                name=f"pts_{pv.b}_{pv.tt}_{gi}")
            nc.tensor.transpose(pts[:, :], pv.trins[:, gi * D:(gi + 1) * D],
                                ident[:, :])
            dsts = _ap(pv.stgt[:, :], [[TT * W, D], [W, 4], [1, QW]],
                       (gi * 4) * W + NQ * QW)
            if gi % 3 == 2:
                nc.scalar.copy(dsts, pts[:, :])
            else:
                nc.vector.tensor_copy(out=dsts, in_=pts[:, :])

    def finish_seg(pv):
        # stg row t = k*9+dy maps to output d = dy*9+k
        for k in range(ND):
            nc.gpsimd.dma_start(
                out=out4[pv.b, k::ND, TT * pv.tt:TT * (pv.tt + 1), :],
                in_=pv.stgt[k * ND:(k + 1) * ND, :])

    # first load: small lead chunks so the first matmuls start early
    nc.gpsimd.dma_start(out=i2h3[0][:, MD:16, MD:MD + W],
                        in_=in2[0, :, 0:12, :])
    nc.gpsimd.dma_start(out=i1h3[0][:, 0:8, :], in_=in1[0, :, 0:8, :])
    nc.vector.memset(i2h3[0][:, 0:MD, MD:MD + W], 0.0)
    nc.gpsimd.dma_start(out=i2h3[0][:, 16:TR, MD:MD + W],
                        in_=in2[0, :, 12:TT + MD, :])
    nc.gpsimd.dma_start(out=i1h3[0][:, 8:TT, :], in_=in1[0, :, 8:TT, :])
    _first_loaded = True
    prev = None
    segs = [(b, tt) for b in range(BPC) for tt in range(NT)]
    for si, (b, tt) in enumerate(segs):
        if si + 1 < len(segs):
            load_seg(*segs[si + 1], (si + 1) % 2)

        gstm = gstm_pool.tile([C, GROW], f16, tag="gstm",
                              name=f"gstm_{b}_{tt}")
        gsts = gsts_pool.tile([C, SROW], f16, tag="gsts",
                              name=f"gsts_{b}_{tt}")
        i13, i2p3 = i1h3[si % 2], i2h3[si % 2]

        pss = None
        for i in range(TT):
            if prev is not None and i >= 14:
                transpose_step(prev, i - 14)
            ps = psm_pool.tile([128, GF], f32, tag="psm",
                               name=f"psm_{b}_{tt}_{i}")
            for q in range(NQ):
                nc.tensor.matmul(
                    out=ps[QW * q:QW * (q + 1), :],
                    lhsT=i13[:, i, QW * q:QW * (q + 1)],
                    rhs=i2p3[:, i:i + ND, QW * q:QW * q + WIN],
                    start=True, stop=True,
                    tile_position=(0, QW * q),
                )
            m = i % 4
            if m == 0:
                pss = pss_pool.tile([128, GF], f32, tag="pss",
                                    name=f"pss_{b}_{tt}_{i}")
            nc.tensor.matmul(
                out=pss[QW * m:QW * (m + 1), :],
                lhsT=i13[:, i, NQ * QW:W],
                rhs=i2p3[:, i:i + ND, NQ * QW:WP],
                start=True, stop=True,
                tile_position=(0, QW * m),
            )
            # strided PSUM read (j outer, dy inner) -> banded
            # gst[x, j*(ND*TT) + yslot*ND + dy] (9-elem write runs; each
            # pixel's band x all yslots stays one contiguous run)
            src_i = _ap(ps[:, :], [[GF, 128], [1, WIN], [WIN, ND]], 0)
            dst_i = _ap(gstm[:, :], [[GROW, 128], [ND * TT, WIN], [1, ND]],
                        i * ND)
            if i % 3 != 2:
                nc.vector.tensor_scalar_mul(dst_i, src_i, inv_c)
            else:
                nc.scalar.mul(dst_i, src_i, inv_c)
            if m == 3:
                gi = i // 4
                src_s = _ap(pss[:, :], [[GF, 128], [1, WIN], [WIN, ND]], 0)
                dst_s = _ap(gsts[:, :], [[SROW, 128], [ND * NG, WIN], [1, ND]],
                            gi * ND)
                if gi % 3 == 2:
                    nc.scalar.mul(dst_s, src_s, inv_c)
                else:
                    nc.vector.tensor_scalar_mul(dst_s, src_s, inv_c)

        if prev is not None:
            for j in range(TT - 14, TT):
                transpose_step(prev, j)
            finish_seg(prev)

        # per-u band extraction (partition stride = whole rows only)
        cur = _Seg()
        cur.b, cur.tt = b, tt
        cur.bnd = bndm_pool.tile([C, BROW], f16, tag="bndm",
                                 name=f"bnd_{b}_{tt}")
        cur.bnds = bnds_pool.tile([C, SBROW], f16, tag="bnds",
                                  name=f"bnds_{b}_{tt}")
        cur.stgt = stg_pool.tile([96, TT * W], f16, tag="stg",
                                 name=f"stg_{b}_{tt}")
        # banded gst: pixel x=32q+u needs j in [u, u+9) -> one contiguous
        # run of ND*(ND*TT) = BROW elems per partition (4 big descriptors)
        for u in range(QW):
            eng = nc.sync if u % 2 == 0 else nc.scalar
            eng.dma_start(
                out=_ap(cur.bnd[:, :], [[QW * BROW, 4], [1, BROW]], u * BROW),
                in_=_ap(gstm[:, :], [[QW * GROW, 4], [1, BROW]],
                        u * GROW + u * ND * TT),
            )
        for u in range(QW):
            eng = nc.scalar if u % 2 == 0 else nc.sync
            eng.dma_start(
                out=_ap(cur.bnds[:, :], [[QW * SBROW, 4], [1, SBROW]],
                        u * SBROW),
                in_=_ap(gsts[:, :], [[QW * SROW, 4], [1, SBROW]],
                        u * SROW + u * ND * NG),
            )
        # reorder bnd (dxi, yslot, dy) -> trin (yslot, t): iterate
        # (yslot, dxi, dy) so writes are fully contiguous, reads 18B runs
        cur.trin = trim_pool.tile([C, BROW], f16, tag="trim",
                                  name=f"trin_{b}_{tt}")
        cur.trins = tris_pool.tile([C, SBROW], f16, tag="trins",
                                   name=f"trins_{b}_{tt}")
        TC = TT // 3
        for e in range(3):
            dst_r = _ap(cur.trin[:, :], [[BROW, 128], [D, TC], [ND, ND],
                                         [1, ND]], e * TC * D)
            src_r = _ap(cur.bnd[:, :], [[BROW, 128], [ND, TC], [ND * TT, ND],
                                        [1, ND]], e * TC * ND)
            if e == 1:
                nc.scalar.copy(dst_r, src_r)
            else:
                nc.vector.tensor_copy(out=dst_r, in_=src_r)
        nc.scalar.copy(
            _ap(cur.trins[:, :], [[SBROW, 128], [D, NG], [ND, ND], [1, ND]],
                0),
            _ap(cur.bnds[:, :], [[SBROW, 128], [ND, NG], [ND * NG, ND],
                                 [1, ND]], 0),
        )
        prev = cur

    for i in range(TT):
        transpose_step(prev, i)
    finish_seg(prev)


_NC_CACHE = {}


def _build_module():
    if "nc" in _NC_CACHE:
        return _NC_CACHE["nc"]
    nc = bass.Bass("TRN2", target_bir_lowering=False, debug=False)
    in1 = nc.dram_tensor("input1", [BPC, C, H, W], f32, kind="ExternalInput").ap()
    in2 = nc.dram_tensor("input2", [BPC, C, H, W], f32, kind="ExternalInput").ap()
    out = nc.dram_tensor("output", [BPC, D, H, W], f16, kind="ExternalOutput").ap()
    with tile.TileContext(nc) as tc:
        with ExitStack() as ctx:
            _corr_kernel(ctx, tc, out, in1, in2)
    _split_waits(nc)
    _NC_CACHE["nc"] = nc
    return nc


def kernel(input1: np.ndarray, input2: np.ndarray, **trace_kwargs) -> np.ndarray:
    input1 = np.ascontiguousarray(input1, dtype=np.float32)
    input2 = np.ascontiguousarray(input2, dtype=np.float32)
    assert input1.shape == (B, C, H, W) and input2.shape == (B, C, H, W)

    nc = _build_module()
    in_maps = []
    for k in range(N_CORES):
        sl = slice(k * BPC, (k + 1) * BPC)
        in_maps.append({
            "input1": input1[sl],
            "input2": input2[sl],
        })
    res = run_bass_kernel_spmd(nc, in_maps, list(range(N_CORES)), **trace_kwargs)
    outs = [res.results[k]["output"] for k in range(N_CORES)]
    full = np.concatenate(outs, axis=0).astype(np.float32)
    if trace_kwargs:
        kernel.last_results = res
    return full



# revision 34
# speedup vs baseline: 1.8229x; 1.0714x over previous
"""FlowNet-style patch correlation (KERNEL=1, MAX_DISP=4, pad=4) on 8 trn2
NeuronCores — Gram-matmul formulation.

Per core (2 batches), per pixel row y:
  - 5 matmuls (4 quadrants of x 0..127 via tile_position, + side x 128..159
    packed 4 consecutive y per bank): lhsT = I1[:, y, 32q:32q+32] fp16,
    rhs = I2pad[:, y:y+9, 32q:32q+40] (all 9 dy in one instruction) ->
    PSUM [128, 360] Gram G[(q,u), (dy,j)] = sum_c I1[c,y,x]*I2[c,y+dy-4,x'].
  - scaled engine copy per bank, reading PSUM in (j, dy) order so the dense
    fp16 SBUF stage gst[p, slot*360 + j*9+dy] is band-interleaved.
  - per-u extraction DMA (partition stride = exactly 32 rows -> BIR-legal)
    gathers the 81 useful values of pixel x = 32q+u (at in-row offset u*9)
    contiguously: band[x, slot*81 + t], t = (dx+4)*9 + (dy+4).
  - PE transpose [128,81] -> PSUM [81,128] per y-slot, engine copy into a
    fp16 staging tile, 9 gpsimd cast-DMAs (one per dx) write out fp32.
Pipelined in 32-row segments with 3-deep gst/band rings: extraction of ring
N overlaps compute of N+1 and N+2; previous ring's transposes interleave
into the current matmul loop with an 8-iteration delay.
"""

import sys
from contextlib import ExitStack

import numpy as np

for _p in ("/opt/trn_rl_repo", "/root/.axon_site/_ro/trn_rl_repo"):
    if _p not in sys.path:
        sys.path.insert(0, _p)

import concourse.bass as bass
import concourse.tile as tile
from concourse import mybir
from concourse.bass_utils import run_bass_kernel_spmd
from concourse.masks import make_identity

B, C, H, W = 16, 128, 96, 160
MD = 4
ND = 2 * MD + 1            # 9
D = ND * ND                # 81
N_CORES = 8
BPC = B // N_CORES         # 2
HP, WP = H + 2 * MD, W + 2 * MD    # 104 x 168
NQ = 4                     # main quadrants (x 0..127)
QW = 32                    # quadrant width
WIN = QW + 2 * MD          # 40 rhs cols per (dy, quadrant)
GF = ND * WIN              # 360 psum free size per bank
TT = 48                    # ys per ring segment
NT = H // TT               # 3 segments per batch
NG = TT // 4               # side groups per segment = 8
TR = TT + 2 * MD           # padded i2 rows per segment tensor = 40

f32 = mybir.dt.float32
f16 = mybir.dt.float16

GROW = TT * GF             # gst main row elems per segment (32*360)
SROW = NG * GF             # gst side row (8*360)
BROW = TT * D              # band main row (32*81)
SBROW = NG * D             # band side row (8*81)


def _split_waits(nc, limit=1):
    n = 0
    for fn in nc.m.functions:
        for blk in fn.blocks:
            out = []
            for inst in blk.instructions:
                si = inst.sync_info
                if si is not None and si.on_wait and len(si.on_wait) > limit:
                    waits = list(si.on_wait)
                    for w in waits[:-limit]:
                        out.append(mybir.InstEventSemaphore(
                            name=nc.get_next_instruction_name(),
                            sync_info=mybir.SyncInfo(on_wait=[w], on_update=[]),
                            engine=inst.engine,
                            ins=[], outs=[],
                        ))
                        n += 1
                    inst.sync_info = mybir.SyncInfo(
                        on_wait=waits[-limit:], on_update=list(si.on_update or []))
                out.append(inst)
            blk.instructions[:] = out
    return n


def _ap(t_ap, dims, offset):
    c = t_ap.copy()
    c.ap = mybir.VecI64Pair(dims)
    c.offset = offset
    return c


class _Seg:
    pass


def _corr_kernel(ctx, tc, out4, in1, in2):
    nc = tc.nc
    const_pool = ctx.enter_context(tc.tile_pool(name="const", bufs=1))
    in_pool = ctx.enter_context(tc.tile_pool(name="in", bufs=1))
    gstm_pool = ctx.enter_context(tc.tile_pool(name="gstm", bufs=2))
    gsts_pool = ctx.enter_context(tc.tile_pool(name="gsts", bufs=2))
    bndm_pool = ctx.enter_context(tc.tile_pool(name="bndm", bufs=1))
    bnds_pool = ctx.enter_context(tc.tile_pool(name="bnds", bufs=1))
    trim_pool = ctx.enter_context(tc.tile_pool(name="trim", bufs=1))
    tris_pool = ctx.enter_context(tc.tile_pool(name="tris", bufs=1))
    stg_pool = ctx.enter_context(tc.tile_pool(name="stg", bufs=2))
    psm_pool = ctx.enter_context(tc.tile_pool(name="psm", bufs=3, space="PSUM"))
    pss_pool = ctx.enter_context(tc.tile_pool(name="pss", bufs=2, space="PSUM"))
    pt_pool = ctx.enter_context(tc.tile_pool(name="pt", bufs=2, space="PSUM"))
    pts_pool = ctx.enter_context(tc.tile_pool(name="pts", bufs=1, space="PSUM"))

    ident = const_pool.tile([128, 128], f16, name="ident")
    make_identity(nc, ident[:, :])

    # double-buffered per-segment inputs; i2 is row-flat (no x padding) with
    # a 4-elem front pad + rear slack so shifted windows stay in-tile; x-edge
    # garbage is zeroed in stgt before the output DMA.
    FP = MD
    I2F = FP + TR * W + 2 * MD + 4
    i1h = [in_pool.tile([C, TT * W], f16, tag=f"i1h{t}", name=f"i1h{t}")
           for t in range(2)]
    i2h = [in_pool.tile([C, I2F], f16, tag=f"i2h{t}", name=f"i2h{t}")
           for t in range(2)]
    i1h3 = [t[:, :].rearrange("p (y x) -> p y x", y=TT) for t in i1h]

    def i2rows(buf, r0, r1):
        return _ap(i2h[buf][:, :], [[I2F, 128], [1, (r1 - r0) * W]],
                   FP + r0 * W)

    inv_c = 1.0 / C

    def load_seg(b, tt, buf):
        for c0, c1 in ((0, 16), (16, TT)):
            nc.gpsimd.dma_start(
                out=i1h3[buf][:, c0:c1, :],
                in_=in1[b, :, TT * tt + c0:TT * tt + c1, :])
        ylo, yhi = TT * tt - MD, TT * (tt + 1) + MD
        # zero the halo rows that fall outside the image
        if ylo < 0:
            nc.vector.memset(i2rows(buf, 0, -ylo), 0.0)
        if yhi > H:
            nc.vector.memset(i2rows(buf, TR - (yhi - H), TR), 0.0)
        for c0, c1 in ((ylo, ylo + 28), (ylo + 28, yhi)):
            clo, chi = max(c0, 0), min(c1, H)
            if clo < chi:
                nc.gpsimd.dma_start(
                    out=i2rows(buf, clo - ylo, chi - ylo),
                    in_=in2[b, :, clo:chi, :])

    def transpose_step(pv, i):
        pt = pt_pool.tile([D, 128], f16, tag="pt",
                          name=f"pt_{pv.b}_{pv.tt}_{i}")
        # trin layout: trin[x, yslot*D + t] -> contiguous weights slice
        nc.tensor.transpose(pt[:, :], pv.trin[:, i * D:(i + 1) * D],
                            ident[:, :])
        if i % 3 == 2:
            nc.scalar.copy(pv.stgt[0:D, i * W:i * W + NQ * QW], pt[:, :])
        else:
            nc.vector.tensor_copy(out=pv.stgt[0:D, i * W:i * W + NQ * QW],
                                  in_=pt[:, :])
        if i % 4 == 3:
            gi = i // 4
            pts = pts_pool.tile([D, 128], f16, tag="pts",
                                name=f"pts_{pv.b}_{pv.tt}_{gi}")
            nc.tensor.transpose(pts[:, :], pv.trins[:, gi * D:(gi + 1) * D],
                                ident[:, :])
            dsts = _ap(pv.stgt[:, :], [[TT * W, D], [W, 4], [1, QW]],
                       (gi * 4) * W + NQ * QW)
            if gi % 3 == 2:
                nc.scalar.copy(dsts, pts[:, :])
            else:
                nc.vector.tensor_copy(out=dsts, in_=pts[:, :])

    def finish_seg(pv):
        # stg row t = k*9+dy maps to output d = dy*9+k
        for k in range(ND):
            nc.gpsimd.dma_start(
                out=out4[pv.b, k::ND, TT * pv.tt:TT * (pv.tt + 1), :],
                in_=pv.stgt[k * ND:(k + 1) * ND, :])

    # first load: small lead chunks so the first matmuls start early
    nc.gpsimd.dma_start(out=i2rows(0, MD, 16), in_=in2[0, :, 0:12, :])
    nc.gpsimd.dma_start(out=i1h3[0][:, 0:8, :], in_=in1[0, :, 0:8, :])
    nc.vector.memset(i2rows(0, 0, MD), 0.0)
    nc.gpsimd.dma_start(out=i2rows(0, 16, TR), in_=in2[0, :, 12:TT + MD, :])
    nc.gpsimd.dma_start(out=i1h3[0][:, 8:TT, :], in_=in1[0, :, 8:TT, :])
    _first_loaded = True
    prev = None
    segs = [(b, tt) for b in range(BPC) for tt in range(NT)]
    for si, (b, tt) in enumerate(segs):
        if si + 1 < len(segs):
            load_seg(*segs[si + 1], (si + 1) % 2)

        gstm = gstm_pool.tile([C, GROW], f16, tag="gstm",
                              name=f"gstm_{b}_{tt}")
        gsts = gsts_pool.tile([C, SROW], f16, tag="gsts",
                              name=f"gsts_{b}_{tt}")
        i13, i2f = i1h3[si % 2], i2h[si % 2][:, :]

        pss = None
        for i in range(TT):
            if prev is not None and i >= 14:
                transpose_step(prev, i - 14)
            ps = psm_pool.tile([128, GF], f32, tag="psm",
                               name=f"psm_{b}_{tt}_{i}")
            for q in range(NQ):
                nc.tensor.matmul(
                    out=ps[QW * q:QW * (q + 1), :],
                    lhsT=i13[:, i, QW * q:QW * (q + 1)],
                    rhs=_ap(i2f, [[I2F, 128], [W, ND], [1, WIN]],
                            FP + i * W + QW * q - MD),
                    start=True, stop=True,
                    tile_position=(0, QW * q),
                )
            m = i % 4
            if m == 0:
                pss = pss_pool.tile([128, GF], f32, tag="pss",
                                    name=f"pss_{b}_{tt}_{i}")
            nc.tensor.matmul(
                out=pss[QW * m:QW * (m + 1), :],
                lhsT=i13[:, i, NQ * QW:W],
                rhs=_ap(i2f, [[I2F, 128], [W, ND], [1, WIN]],
                        FP + i * W + NQ * QW - MD),
                start=True, stop=True,
                tile_position=(0, QW * m),
            )
            # strided PSUM read (j outer, dy inner) -> banded
            # gst[x, j*(ND*TT) + yslot*ND + dy] (9-elem write runs; each
            # pixel's band x all yslots stays one contiguous run)
            src_i = _ap(ps[:, :], [[GF, 128], [1, WIN], [WIN, ND]], 0)
            dst_i = _ap(gstm[:, :], [[GROW, 128], [ND * TT, WIN], [1, ND]],
                        i * ND)
            if i % 3 != 2:
                nc.vector.tensor_scalar_mul(dst_i, src_i, inv_c)
            else:
                nc.scalar.mul(dst_i, src_i, inv_c)
            if m == 3:
                gi = i // 4
                src_s = _ap(pss[:, :], [[GF, 128], [1, WIN], [WIN, ND]], 0)
                dst_s = _ap(gsts[:, :], [[SROW, 128], [ND * NG, WIN], [1, ND]],
                            gi * ND)
                if gi % 3 == 2:
                    nc.scalar.mul(dst_s, src_s, inv_c)
                else:
                    nc.vector.tensor_scalar_mul(dst_s, src_s, inv_c)

        if prev is not None:
            for j in range(TT - 14, TT):
                transpose_step(prev, j)
            finish_seg(prev)

        # per-u band extraction (partition stride = whole rows only)
        cur = _Seg()
        cur.b, cur.tt = b, tt
        cur.bnd = bndm_pool.tile([C, BROW], f16, tag="bndm",
                                 name=f"bnd_{b}_{tt}")
        cur.bnds = bnds_pool.tile([C, SBROW], f16, tag="bnds",
                                  name=f"bnds_{b}_{tt}")
        cur.stgt = stg_pool.tile([96, TT * W], f16, tag="stg",
                                 name=f"stg_{b}_{tt}")
        # zero x-edge garbage (unpadded i2 reads): q=0 pixels' j<MD columns
        # (x2<0) and side pixels' j>=36 columns (x2>=W)
        nc.vector.memset(
            _ap(gstm[:, :], [[GROW, QW], [1, MD * ND * TT]], 0), 0.0)
        nc.vector.memset(
            _ap(gsts[:, :], [[SROW, 128], [1, MD * ND * NG]],
                (WIN - MD) * ND * NG), 0.0)
        # banded gst: pixel x=32q+u needs j in [u, u+9) -> one contiguous
        # run of BROW elems per partition (4 big descriptors per DMA)
        for u in range(QW):
            nc.sync.dma_start(
                out=_ap(cur.bnd[:, :], [[QW * BROW, 4], [1, BROW]], u * BROW),
                in_=_ap(gstm[:, :], [[QW * GROW, 4], [1, BROW]],
                        u * GROW + u * ND * TT),
            )
        for u in range(QW):
            eng = nc.scalar if u % 2 == 0 else nc.gpsimd
            eng.dma_start(
                out=_ap(cur.bnds[:, :], [[QW * SBROW, 4], [1, SBROW]],
                        u * SBROW),
                in_=_ap(gsts[:, :], [[QW * SROW, 4], [1, SBROW]],
                        u * SROW + u * ND * NG),
            )
        # reorder bnd (dxi, yslot, dy) -> trin (yslot, t): iterate
        # (yslot, dxi, dy) so writes are fully contiguous, reads 18B runs
        cur.trin = trim_pool.tile([C, BROW], f16, tag="trim",
                                  name=f"trin_{b}_{tt}")
        cur.trins = tris_pool.tile([C, SBROW], f16, tag="trins",
                                   name=f"trins_{b}_{tt}")
        TC = TT // 3
        for e in range(3):
            dst_r = _ap(cur.trin[:, :], [[BROW, 128], [D, TC], [ND, ND],
                                         [1, ND]], e * TC * D)
            src_r = _ap(cur.bnd[:, :], [[BROW, 128], [ND, TC], [ND * TT, ND],
                                        [1, ND]], e * TC * ND)
            if e == 1:
                nc.scalar.copy(dst_r, src_r)
            else:
                nc.vector.tensor_copy(out=dst_r, in_=src_r)
        nc.scalar.copy(
            _ap(cur.trins[:, :], [[SBROW, 128], [D, NG], [ND, ND], [1, ND]],
                0),
            _ap(cur.bnds[:, :], [[SBROW, 128], [ND, NG], [ND * NG, ND],
                                 [1, ND]], 0),
        )
        prev = cur

    for i in range(TT):
        transpose_step(prev, i)
    finish_seg(prev)


_NC_CACHE = {}


def _build_module():
    if "nc" in _NC_CACHE:
        return _NC_CACHE["nc"]
    nc = bass.Bass("TRN2", target_bir_lowering=False, debug=False)
    in1 = nc.dram_tensor("input1", [BPC, C, H, W], f32, kind="ExternalInput").ap()
    in2 = nc.dram_tensor("input2", [BPC, C, H, W], f32, kind="ExternalInput").ap()
    out = nc.dram_tensor("output", [BPC, D, H, W], f16, kind="ExternalOutput").ap()
    with tile.TileContext(nc) as tc:
        with ExitStack() as ctx:
            _corr_kernel(ctx, tc, out, in1, in2)
    _split_waits(nc)
    _NC_CACHE["nc"] = nc
    return nc


def kernel(input1: np.ndarray, input2: np.ndarray, **trace_kwargs) -> np.ndarray:
    input1 = np.ascontiguousarray(input1, dtype=np.float32)
    input2 = np.ascontiguousarray(input2, dtype=np.float32)
    assert input1.shape == (B, C, H, W) and input2.shape == (B, C, H, W)

    nc = _build_module()
    in_maps = []
    for k in range(N_CORES):
        sl = slice(k * BPC, (k + 1) * BPC)
        in_maps.append({
            "input1": input1[sl],
            "input2": input2[sl],
        })
    res = run_bass_kernel_spmd(nc, in_maps, list(range(N_CORES)), **trace_kwargs)
    outs = [res.results[k]["output"] for k in range(N_CORES)]
    full = np.concatenate(outs, axis=0).astype(np.float32)
    if trace_kwargs:
        kernel.last_results = res
    return full



# revision 42
# speedup vs baseline: 1.9981x; 1.0961x over previous
"""FlowNet-style patch correlation (KERNEL=1, MAX_DISP=4, pad=4) on 8 trn2
NeuronCores — Gram-matmul formulation.

Per core (2 batches), per pixel row y:
  - 5 matmuls (4 quadrants of x 0..127 via tile_position, + side x 128..159
    packed 4 consecutive y per bank): lhsT = I1[:, y, 32q:32q+32] fp16,
    rhs = I2pad[:, y:y+9, 32q:32q+40] (all 9 dy in one instruction) ->
    PSUM [128, 360] Gram G[(q,u), (dy,j)] = sum_c I1[c,y,x]*I2[c,y+dy-4,x'].
  - scaled engine copy per bank, reading PSUM in (j, dy) order so the dense
    fp16 SBUF stage gst[p, slot*360 + j*9+dy] is band-interleaved.
  - per-u extraction DMA (partition stride = exactly 32 rows -> BIR-legal)
    gathers the 81 useful values of pixel x = 32q+u (at in-row offset u*9)
    contiguously: band[x, slot*81 + t], t = (dx+4)*9 + (dy+4).
  - PE transpose [128,81] -> PSUM [81,128] per y-slot, engine copy into a
    fp16 staging tile, 9 gpsimd cast-DMAs (one per dx) write out fp32.
Pipelined in 32-row segments with 3-deep gst/band rings: extraction of ring
N overlaps compute of N+1 and N+2; previous ring's transposes interleave
into the current matmul loop with an 8-iteration delay.
"""

import sys
from contextlib import ExitStack

import numpy as np

for _p in ("/opt/trn_rl_repo", "/root/.axon_site/_ro/trn_rl_repo"):
    if _p not in sys.path:
        sys.path.insert(0, _p)

import concourse.bass as bass
import concourse.tile as tile
from concourse import mybir
from concourse.bass_utils import run_bass_kernel_spmd
from concourse.masks import make_identity

B, C, H, W = 16, 128, 96, 160
MD = 4
ND = 2 * MD + 1            # 9
D = ND * ND                # 81
N_CORES = 8
BPC = B // N_CORES         # 2
HP, WP = H + 2 * MD, W + 2 * MD    # 104 x 168
NQ = 4                     # main quadrants (x 0..127)
QW = 32                    # quadrant width
WIN = QW + 2 * MD          # 40 rhs cols per (dy, quadrant)
GF = ND * WIN              # 360 psum free size per bank
TT = 48                    # ys per ring segment
NT = H // TT               # 3 segments per batch
NG = TT // 4               # side groups per segment = 8
TR = TT + 2 * MD           # padded i2 rows per segment tensor = 40

f32 = mybir.dt.float32
f16 = mybir.dt.float16

GROW = TT * GF             # gst main row elems per segment (32*360)
SROW = NG * GF             # gst side row (8*360)
BROW = TT * D              # band main row (32*81)
SBROW = NG * D             # band side row (8*81)


def _split_waits(nc, limit=1):
    n = 0
    for fn in nc.m.functions:
        for blk in fn.blocks:
            out = []
            for inst in blk.instructions:
                si = inst.sync_info
                if si is not None and si.on_wait and len(si.on_wait) > limit:
                    waits = list(si.on_wait)
                    for w in waits[:-limit]:
                        out.append(mybir.InstEventSemaphore(
                            name=nc.get_next_instruction_name(),
                            sync_info=mybir.SyncInfo(on_wait=[w], on_update=[]),
                            engine=inst.engine,
                            ins=[], outs=[],
                        ))
                        n += 1
                    inst.sync_info = mybir.SyncInfo(
                        on_wait=waits[-limit:], on_update=list(si.on_update or []))
                out.append(inst)
            blk.instructions[:] = out
    return n


def _ap(t_ap, dims, offset):
    c = t_ap.copy()
    c.ap = mybir.VecI64Pair(dims)
    c.offset = offset
    return c


class _Seg:
    pass


def _corr_kernel(ctx, tc, out4, in1, in2):
    nc = tc.nc
    const_pool = ctx.enter_context(tc.tile_pool(name="const", bufs=1))
    in_pool = ctx.enter_context(tc.tile_pool(name="in", bufs=1))
    gstm_pool = ctx.enter_context(tc.tile_pool(name="gstm", bufs=2))
    gsts_pool = ctx.enter_context(tc.tile_pool(name="gsts", bufs=2))
    bndm_pool = ctx.enter_context(tc.tile_pool(name="bndm", bufs=1))
    bnds_pool = ctx.enter_context(tc.tile_pool(name="bnds", bufs=1))
    trim_pool = ctx.enter_context(tc.tile_pool(name="trim", bufs=1))
    tris_pool = ctx.enter_context(tc.tile_pool(name="tris", bufs=1))
    stg_pool = ctx.enter_context(tc.tile_pool(name="stg", bufs=2))
    psm_pool = ctx.enter_context(tc.tile_pool(name="psm", bufs=3, space="PSUM"))
    pss_pool = ctx.enter_context(tc.tile_pool(name="pss", bufs=2, space="PSUM"))
    pt_pool = ctx.enter_context(tc.tile_pool(name="pt", bufs=2, space="PSUM"))
    pts_pool = ctx.enter_context(tc.tile_pool(name="pts", bufs=1, space="PSUM"))

    ident = const_pool.tile([128, 128], f16, name="ident")
    make_identity(nc, ident[:, :])

    # double-buffered per-segment inputs; i2 is row-flat (no x padding) with
    # a 4-elem front pad + rear slack so shifted windows stay in-tile; x-edge
    # garbage is zeroed in stgt before the output DMA.
    FP = MD
    I2F = FP + TR * W + 2 * MD + 4
    i1h = [in_pool.tile([C, TT * W], f16, tag=f"i1h{t}", name=f"i1h{t}")
           for t in range(2)]
    i2h = [in_pool.tile([C, I2F], f16, tag=f"i2h{t}", name=f"i2h{t}")
           for t in range(2)]
    i1h3 = [t[:, :].rearrange("p (y x) -> p y x", y=TT) for t in i1h]

    def i2rows(buf, r0, r1):
        return _ap(i2h[buf][:, :], [[I2F, 128], [1, (r1 - r0) * W]],
                   FP + r0 * W)

    inv_c = 1.0 / C

    def load_seg(b, tt, buf):
        for c0, c1 in ((0, 16), (16, TT)):
            nc.gpsimd.dma_start(
                out=i1h3[buf][:, c0:c1, :],
                in_=in1[b, :, TT * tt + c0:TT * tt + c1, :])
        ylo, yhi = TT * tt - MD, TT * (tt + 1) + MD
        # zero the halo rows that fall outside the image
        if ylo < 0:
            nc.vector.memset(i2rows(buf, 0, -ylo), 0.0)
        if yhi > H:
            nc.vector.memset(i2rows(buf, TR - (yhi - H), TR), 0.0)
        for c0, c1 in ((ylo, ylo + 28), (ylo + 28, yhi)):
            clo, chi = max(c0, 0), min(c1, H)
            if clo < chi:
                nc.gpsimd.dma_start(
                    out=i2rows(buf, clo - ylo, chi - ylo),
                    in_=in2[b, :, clo:chi, :])

    def transpose_main(pv, i):
        pt = pt_pool.tile([D, 128], f16, tag="pt",
                          name=f"pt_{pv.b}_{pv.tt}_{i}")
        # trin layout: trin[x, yslot*D + t] -> contiguous weights slice
        nc.tensor.transpose(pt[:, :], pv.trin[:, i * D:(i + 1) * D],
                            ident[:, :])
        if i % 3 == 2:
            nc.scalar.copy(pv.stgt[0:D, i * W:i * W + NQ * QW], pt[:, :])
        else:
            nc.vector.tensor_copy(out=pv.stgt[0:D, i * W:i * W + NQ * QW],
                                  in_=pt[:, :])

    def transpose_side(pv, gi):
        pts = pts_pool.tile([D, 128], f16, tag="pts",
                            name=f"pts_{pv.b}_{pv.tt}_{gi}")
        nc.tensor.transpose(pts[:, :], pv.trins[:, gi * D:(gi + 1) * D],
                            ident[:, :])
        dsts = _ap(pv.stgt[:, :], [[TT * W, D], [W, 4], [1, QW]],
                   (gi * 4) * W + NQ * QW)
        if gi % 3 == 2:
            nc.scalar.copy(dsts, pts[:, :])
        else:
            nc.vector.tensor_copy(out=dsts, in_=pts[:, :])

    def finish_seg(pv):
        # stg row t = k*9+dy maps to output d = dy*9+k
        for k in range(ND):
            nc.gpsimd.dma_start(
                out=out4[pv.b, k::ND, TT * pv.tt:TT * (pv.tt + 1), :],
                in_=pv.stgt[k * ND:(k + 1) * ND, :])

    # first load: small lead chunks so the first matmuls start early
    nc.gpsimd.dma_start(out=i2rows(0, MD, 16), in_=in2[0, :, 0:12, :])
    nc.gpsimd.dma_start(out=i1h3[0][:, 0:8, :], in_=in1[0, :, 0:8, :])
    nc.vector.memset(i2rows(0, 0, MD), 0.0)
    nc.gpsimd.dma_start(out=i2rows(0, 16, TR), in_=in2[0, :, 12:TT + MD, :])
    nc.gpsimd.dma_start(out=i1h3[0][:, 8:TT, :], in_=in1[0, :, 8:TT, :])
    _first_loaded = True
    prev = None
    segs = [(b, tt) for b in range(BPC) for tt in range(NT)]
    for si, (b, tt) in enumerate(segs):
        if si + 1 < len(segs):
            load_seg(*segs[si + 1], (si + 1) % 2)

        gstm = gstm_pool.tile([C, GROW], f16, tag="gstm",
                              name=f"gstm_{b}_{tt}")
        gsts = gsts_pool.tile([C, SROW], f16, tag="gsts",
                              name=f"gsts_{b}_{tt}")
        i13, i2f = i1h3[si % 2], i2h[si % 2][:, :]

        pss = None
        for i in range(TT):
            # prev's transposes start once its extraction+reorder completed
            # (~y=30); 48 main spread over slots 34..47, 12 side over 36..47
            if prev is not None and i >= 34:
                k = i - 34
                lo = 4 * k if k < 6 else 24 + 3 * (k - 6)
                hi = lo + (4 if k < 6 else 3)
                for ti in range(lo, hi):
                    transpose_main(prev, ti)
                if i >= 36:
                    transpose_side(prev, i - 36)
            ps = psm_pool.tile([128, GF], f32, tag="psm",
                               name=f"psm_{b}_{tt}_{i}")
            for q in range(NQ):
                nc.tensor.matmul(
                    out=ps[QW * q:QW * (q + 1), :],
                    lhsT=i13[:, i, QW * q:QW * (q + 1)],
                    rhs=_ap(i2f, [[I2F, 128], [W, ND], [1, WIN]],
                            FP + i * W + QW * q - MD),
                    start=True, stop=True,
                    tile_position=(0, QW * q),
                )
            m = i % 4
            if m == 0:
                pss = pss_pool.tile([128, GF], f32, tag="pss",
                                    name=f"pss_{b}_{tt}_{i}")
            nc.tensor.matmul(
                out=pss[QW * m:QW * (m + 1), :],
                lhsT=i13[:, i, NQ * QW:W],
                rhs=_ap(i2f, [[I2F, 128], [W, ND], [1, WIN]],
                        FP + i * W + NQ * QW - MD),
                start=True, stop=True,
                tile_position=(0, QW * m),
            )
            # strided PSUM read (j outer, dy inner) -> banded
            # gst[x, j*(ND*TT) + yslot*ND + dy] (9-elem write runs; each
            # pixel's band x all yslots stays one contiguous run)
            src_i = _ap(ps[:, :], [[GF, 128], [1, WIN], [WIN, ND]], 0)
            dst_i = _ap(gstm[:, :], [[GROW, 128], [ND * TT, WIN], [1, ND]],
                        i * ND)
            if i % 3 != 2:
                nc.vector.tensor_scalar_mul(dst_i, src_i, inv_c)
            else:
                nc.scalar.mul(dst_i, src_i, inv_c)
            if m == 3:
                gi = i // 4
                src_s = _ap(pss[:, :], [[GF, 128], [1, WIN], [WIN, ND]], 0)
                dst_s = _ap(gsts[:, :], [[SROW, 128], [ND * NG, WIN], [1, ND]],
                            gi * ND)
                if gi % 3 == 2:
                    nc.scalar.mul(dst_s, src_s, inv_c)
                else:
                    nc.vector.tensor_scalar_mul(dst_s, src_s, inv_c)

        if prev is not None:
            finish_seg(prev)

        # per-u band extraction (partition stride = whole rows only)
        cur = _Seg()
        cur.b, cur.tt = b, tt
        cur.bnd = bndm_pool.tile([C, BROW], f16, tag="bndm",
                                 name=f"bnd_{b}_{tt}")
        cur.bnds = bnds_pool.tile([C, SBROW], f16, tag="bnds",
                                  name=f"bnds_{b}_{tt}")
        cur.stgt = stg_pool.tile([96, TT * W], f16, tag="stg",
                                 name=f"stg_{b}_{tt}")
        # zero x-edge garbage (unpadded i2 reads): q=0 pixels' j<MD columns
        # (x2<0) and side pixels' j>=36 columns (x2>=W)
        nc.vector.memset(
            _ap(gstm[:, :], [[GROW, QW], [1, MD * ND * TT]], 0), 0.0)
        nc.vector.memset(
            _ap(gsts[:, :], [[SROW, 128], [1, MD * ND * NG]],
                (WIN - MD) * ND * NG), 0.0)
        # banded gst: pixel x=32q+u needs j in [u, u+9) -> one contiguous
        # run of BROW elems per partition (4 big descriptors per DMA)
        for u in range(QW):
            nc.sync.dma_start(
                out=_ap(cur.bnd[:, :], [[QW * BROW, 4], [1, BROW]], u * BROW),
                in_=_ap(gstm[:, :], [[QW * GROW, 4], [1, BROW]],
                        u * GROW + u * ND * TT),
            )
        for u in range(QW):
            nc.gpsimd.dma_start(
                out=_ap(cur.bnds[:, :], [[QW * SBROW, 4], [1, SBROW]],
                        u * SBROW),
                in_=_ap(gsts[:, :], [[QW * SROW, 4], [1, SBROW]],
                        u * SROW + u * ND * NG),
            )
        # reorder bnd (dxi, yslot, dy) -> trin (yslot, t): iterate
        # (yslot, dxi, dy) so writes are fully contiguous, reads 18B runs
        cur.trin = trim_pool.tile([C, BROW], f16, tag="trim",
                                  name=f"trin_{b}_{tt}")
        cur.trins = tris_pool.tile([C, SBROW], f16, tag="trins",
                                   name=f"trins_{b}_{tt}")
        TC = TT // 3
        for e in range(3):
            dst_r = _ap(cur.trin[:, :], [[BROW, 128], [D, TC], [ND, ND],
                                         [1, ND]], e * TC * D)
            src_r = _ap(cur.bnd[:, :], [[BROW, 128], [ND, TC], [ND * TT, ND],
                                        [1, ND]], e * TC * ND)
            if e == 1:
                nc.scalar.copy(dst_r, src_r)
            else:
                nc.vector.tensor_copy(out=dst_r, in_=src_r)
        nc.scalar.copy(
            _ap(cur.trins[:, :], [[SBROW, 128], [D, NG], [ND, ND], [1, ND]],
                0),
            _ap(cur.bnds[:, :], [[SBROW, 128], [ND, NG], [ND * NG, ND],
                                 [1, ND]], 0),
        )
        prev = cur

    for i in range(TT):
        transpose_main(prev, i)
        if i % 4 == 3:
            transpose_side(prev, i // 4)
    finish_seg(prev)


_NC_CACHE = {}


def _build_module():
    if "nc" in _NC_CACHE:
        return _NC_CACHE["nc"]
    nc = bass.Bass("TRN2", target_bir_lowering=False, debug=False)
    in1 = nc.dram_tensor("input1", [BPC, C, H, W], f32, kind="ExternalInput").ap()
    in2 = nc.dram_tensor("input2", [BPC, C, H, W], f32, kind="ExternalInput").ap()
    out = nc.dram_tensor("output", [BPC, D, H, W], f16, kind="ExternalOutput").ap()
    with tile.TileContext(nc) as tc:
        with ExitStack() as ctx:
            _corr_kernel(ctx, tc, out, in1, in2)
    _split_waits(nc)
    _NC_CACHE["nc"] = nc
    return nc


def kernel(input1: np.ndarray, input2: np.ndarray, **trace_kwargs) -> np.ndarray:
    input1 = np.ascontiguousarray(input1, dtype=np.float32)
    input2 = np.ascontiguousarray(input2, dtype=np.float32)
    assert input1.shape == (B, C, H, W) and input2.shape == (B, C, H, W)

    nc = _build_module()
    in_maps = []
    for k in range(N_CORES):
        sl = slice(k * BPC, (k + 1) * BPC)
        in_maps.append({
            "input1": input1[sl],
            "input2": input2[sl],
        })
    res = run_bass_kernel_spmd(nc, in_maps, list(range(N_CORES)), **trace_kwargs)
    outs = [res.results[k]["output"] for k in range(N_CORES)]
    full = np.concatenate(outs, axis=0).astype(np.float32)
    if trace_kwargs:
        kernel.last_results = res
    return full

